# revision 26
# baseline (speedup 1.0000x reference)
"""Trainium2 Bass kernel for the DLSMN layer (read/compute/write memory layer).

Sharding: 8 cores = 4 batches x 2 sequence-halves. Each core handles 2048
tokens of one batch. The linear-attention kv/ksum statistics are summed over
the full sequence via a pairwise AllReduce. The cache write phase (first
T=256 tokens) is computed on even cores; odd cores' write outputs are
discarded host-side.

Layouts: "fm" = feature-major [feat, tokens] (matmul operand layout),
"tm" = token-major [tokens, feat] (natural / DVE-scalar layout).
All matmul operands are bf16 (fp32 PSUM accumulate); vector math is fp32.
"""

import numpy as np
import ml_dtypes
from contextlib import ExitStack

# ---- model constants (hardcoded per problem spec) ----
B, S, D = 4, 4096, 1024
H, HD = 16, 64
DC, K, L, LAYER = 512, 64, 12, 3
M = L * K              # 768 cache rows
T = 256                # write tokens
SL = S // 2            # 2048 tokens per core
CH = 512               # token chunk
NCH = SL // CH         # 4
ND = D // 128          # 8
NC4 = DC // 128        # 4
NM = M // 128          # 6
DFF = 4 * D            # 4096
NF = DFF // 128        # 32
N_CORES = 8
EPS = 1e-6
LN_EPS = 1e-5
ISQ = 1.0 / float(np.sqrt(np.float32(DC)))

BF = None  # set lazily to mybir.dt.bfloat16
F32 = None

_CACHE = {}


def _build(debug=False, phases=6, timing=False):
    import concourse.bass as bass
    import concourse.tile as tile
    from concourse import bacc, mybir
    from concourse.masks import make_identity

    global BF, F32
    BF = mybir.dt.bfloat16
    F32 = mybir.dt.float32
    AO = mybir.AluOpType
    AF = mybir.ActivationFunctionType

    nc = bacc.Bacc("TRN2", target_bir_lowering=False, debug=False,
                   num_devices=N_CORES)

    def din(name, shape, dt=None):
        return nc.dram_tensor(name, shape, dt or BF, kind="ExternalInput").ap()

    def dout(name, shape, dt):
        return nc.dram_tensor(name, shape, dt, kind="ExternalOutput").ap()

    # ---- external inputs (per-core) ----
    x_tm = din("x_tm", [SL, D], F32)
    x_fm = din("x_fm", [D, SL])
    cacheT = din("cacheT", [DC, M])      # rhs-form cache.T
    cache_m = din("cache_m", [M, DC])    # lhsT-form cache
    localT = din("localT", [DC, K])
    local_f = din("local_f", [K, DC], F32)
    wqr = din("wqr", [D, DC])
    wor = din("wor", [DC, D])
    wgr = din("wgr", [D, 1])
    wq = din("wq", [D, D])
    wk = din("wk", [D, D])
    wv = din("wv", [D, D])
    wo = din("wo", [D, D])
    w1 = din("w1", [D, DFF])             # pre-scaled by ln0_g
    w2 = din("w2", [DFF, D])
    wslot = din("wslot", [D, DC])
    wvw = din("wvw", [D, DC])
    wgw = din("wgw", [D, 1])
    bg_r = din("bg_r", [1], F32)
    bgw = din("bgw", [1], F32)
    b1p = din("b1p", [DFF], F32)         # b1 + ln0_b @ W1
    b2v = din("b2v", [D], F32)
    n1g = din("n1g", [D], F32)
    n1b = din("n1b", [D], F32)
    n2g = din("n2g", [D], F32)
    n2b = din("n2b", [D], F32)

    # ---- external outputs ----
    y_out = dout("y", [SL, D], F32)
    nl_out = dout("new_local", [K, DC], F32)
    dbg = {}
    if debug:
        dbg["dbg_xf"] = dout("dbg_xf", [SL, D], F32)
        dbg["dbg_kv"] = dout("dbg_kv", [64, H * 65], F32)
        dbg["dbg_h"] = dout("dbg_h", [D, SL], F32)

    def fm(ap):   # [BIG, S] dram -> [128, blocks, S]
        return ap.rearrange("(a p) s -> p a s", p=128)

    def tmr(ap):  # [rows, D] dram -> [128, rows/128, D]
        return ap.rearrange("(b p) d -> p b d", p=128)

    with tile.TileContext(nc) as tc, ExitStack() as octx:
        # ---------- persistent constants ----------
        konst = octx.enter_context(tc.tile_pool(name="konst", bufs=1))
        ident = konst.tile([128, 128], BF)
        make_identity(nc, ident)
        ones_bf = konst.tile([128, 1], BF)
        nc.vector.memset(ones_bf, 1.0)
        eps5 = konst.tile([128, 1], F32)
        nc.vector.memset(eps5, LN_EPS)

        def bcast_vec(ap_, n, name, pool=None):
            t = (pool or konst).tile([128, n], F32, name=name)
            src = bass.AP(tensor=ap_.tensor, offset=ap_.offset,
                          ap=[[0, 128]] + list(ap_.ap))
            nc.gpsimd.dma_start(out=t, in_=src)
            return t

        bgr_t = bcast_vec(bg_r, 1, "bgr_t")
        bgw_t = bcast_vec(bgw, 1, "bgw_t")
        b1p_sb = konst.tile([128, NF], F32)
        nc.sync.dma_start(out=b1p_sb, in_=b1p.rearrange("(a p) -> p a", p=128))

        # resident activations with explicit lifetimes (stack order: kvp under xfp)
        # kv tiles use 64 partitions with heads along the free dim: PE matmul
        # operands must start at partition offset 0.
        kvp_pool = tc.alloc_tile_pool(name="kvp", bufs=1)
        kv_acc = kvp_pool.tile([64, H, HD + 1], F32)
        kv_bf = kvp_pool.tile([64, H, HD + 1], BF)
        xfp = tc.alloc_tile_pool(name="xfp", bufs=1)
        xf_fm = xfp.tile([128, ND, SL], BF)          # x_fused feature-major

        # dram spill tiles (tracked by Tile)
        dsp = octx.enter_context(tc.tile_pool(name="dsp", bufs=1, space="DRAM"))
        xf_tm_d = dsp.tile([SL, D], F32)
        pq_d = dsp.tile([D, SL], BF)
        h_d = dsp.tile([D, SL], BF)
        x1_d = dsp.tile([SL, D], BF)
        h1_d = dsp.tile([DFF, SL], BF)
        kv_part = dsp.tile([64, H * (HD + 1)], F32)
        kv_sum = dsp.tile([64, H * (HD + 1)], F32)

        # =======================================================
        # PHASE R: gated cross-attention read over the cache
        # =======================================================
        with ExitStack() as ctx:
            wp = ctx.enter_context(tc.tile_pool(name="wR", bufs=1))
            wqr_sb = wp.tile([128, ND, DC], BF)
            nc.sync.dma_start(out=wqr_sb, in_=fm(wqr))
            cT_sb = wp.tile([128, NC4, M], BF)
            nc.sync.dma_start(out=cT_sb, in_=fm(cacheT))
            cm_sb = wp.tile([128, NM, DC], BF)
            nc.sync.dma_start(out=cm_sb, in_=fm(cache_m))
            wor_sb = wp.tile([128, NC4, D], BF)
            nc.sync.dma_start(out=wor_sb, in_=fm(wor))
            wgr_sb = wp.tile([128, ND, 1], BF)
            nc.sync.dma_start(out=wgr_sb, in_=fm(wgr))

            io = ctx.enter_context(tc.tile_pool(name="ioR", bufs=2))
            tmp = ctx.enter_context(tc.tile_pool(name="tmpR", bufs=2))
            sm = ctx.enter_context(tc.tile_pool(name="smR", bufs=4))
            psA = ctx.enter_context(tc.tile_pool(name="psA_R", bufs=2, space="PSUM"))
            psB = ctx.enter_context(tc.tile_pool(name="psB_R", bufs=1, space="PSUM"))
            psC = ctx.enter_context(tc.tile_pool(name="psC_R", bufs=2, space="PSUM"))
            psT = ctx.enter_context(tc.tile_pool(name="psT_R", bufs=2, space="PSUM"))

            for ci in range(NCH):
                c0 = ci * CH
                xfm_ch = io.tile([128, ND, CH], BF, tag="xfm")
                nc.sync.dma_start(out=xfm_ch, in_=fm(x_fm)[:, :, c0:c0 + CH])
                xtm_ch = io.tile([128, CH // 128, D], F32, tag="xtm")
                nc.sync.dma_start(out=xtm_ch, in_=tmr(x_tm[c0:c0 + CH, :]))

                # q_r^T = Wq_r^T @ x^T   [DC, CH]
                qr_sb = tmp.tile([128, NC4, CH], BF, tag="qr")
                for mb in range(NC4):
                    ps = psA.tile([128, CH], F32, tag="psA")
                    for kc in range(ND):
                        nc.tensor.matmul(ps, wqr_sb[:, kc, mb * 128:(mb + 1) * 128],
                                         xfm_ch[:, kc, :],
                                         start=(kc == 0), stop=(kc == ND - 1))
                    nc.scalar.copy(qr_sb[:, mb, :], ps)

                # scores^T = cache @ q_r^T -> exp  [M, CH]
                es_sb = tmp.tile([128, NM, CH], BF, tag="es")
                for mb in range(NM):
                    ps = psA.tile([128, CH], F32, tag="psA")
                    for cc in range(NC4):
                        nc.tensor.matmul(ps, cT_sb[:, cc, mb * 128:(mb + 1) * 128],
                                         qr_sb[:, cc, :],
                                         start=(cc == 0), stop=(cc == NC4 - 1))
                    nc.scalar.activation(es_sb[:, mb, :], ps, AF.Exp, scale=ISQ)

                # per-token gate*recip(den)
                gs_sb = sm.tile([128, CH // 128, 1], F32, tag="gs")
                for sub in range(CH // 128):
                    s0 = sub * 128
                    dps = psC.tile([128, 1], F32, tag="psC")
                    for mb in range(NM):
                        nc.tensor.matmul(dps, es_sb[:, mb, s0:s0 + 128], ones_bf,
                                         start=(mb == 0), stop=(mb == NM - 1))
                    gps = psC.tile([128, 1], F32, tag="psC")
                    for kc in range(ND):
                        nc.tensor.matmul(gps, xfm_ch[:, kc, s0:s0 + 128],
                                         wgr_sb[:, kc, :],
                                         start=(kc == 0), stop=(kc == ND - 1))
                    rd = sm.tile([128, 1], F32, tag="rd")
                    nc.vector.reciprocal(rd, dps)
                    sg = sm.tile([128, 1], F32, tag="sg")
                    nc.scalar.activation(sg, gps, AF.Sigmoid, bias=bgr_t)
                    nc.vector.tensor_mul(gs_sb[:, sub, :], rd, sg)

                # ctx_raw^T = cache^T @ exp  [DC, CH]
                ctxT_sb = tmp.tile([128, NC4, CH], BF, tag="ctxT")
                for cb in range(NC4):
                    ps = psA.tile([128, CH], F32, tag="psA")
                    for mc in range(NM):
                        nc.tensor.matmul(ps, cm_sb[:, mc, cb * 128:(cb + 1) * 128],
                                         es_sb[:, mc, :],
                                         start=(mc == 0), stop=(mc == NM - 1))
                    nc.scalar.copy(ctxT_sb[:, cb, :], ps)

                # x_fused = x + gscale * (ctx_raw @ Wo_r)   (token-major)
                xft_sb = io.tile([128, CH // 128, D], F32, tag="xft")
                for sub in range(CH // 128):
                    s0 = sub * 128
                    wps = psB.tile([128, D], F32, tag="psB")
                    for nh in range(2):
                        for cc in range(NC4):
                            nc.tensor.matmul(wps[:, nh * 512:(nh + 1) * 512],
                                             ctxT_sb[:, cc, s0:s0 + 128],
                                             wor_sb[:, cc, nh * 512:(nh + 1) * 512],
                                             start=(cc == 0), stop=(cc == NC4 - 1))
                    nc.vector.scalar_tensor_tensor(
                        out=xft_sb[:, sub, :], in0=wps, scalar=gs_sb[:, sub, :],
                        in1=xtm_ch[:, sub, :], op0=AO.mult, op1=AO.add)
                nc.sync.dma_start(out=tmr(xf_tm_d[c0:c0 + CH, :]), in_=xft_sb)

                # transpose into resident feature-major copy
                for sub in range(CH // 128):
                    xfb = tmp.tile([128, D], BF, tag="xfb")
                    nc.scalar.copy(xfb, xft_sb[:, sub, :])
                    for db in range(ND):
                        tp = psT.tile([128, 128], BF, tag="psT")
                        nc.tensor.transpose(tp, xfb[:, db * 128:(db + 1) * 128], ident)
                        nc.vector.tensor_copy(
                            xf_fm[:, db, c0 + sub * 128:c0 + (sub + 1) * 128], tp)

        if debug:
            nc.sync.dma_start(out=tmr(dbg["dbg_xf"]), in_=tmr(xf_tm_d[:, :]))
        if False:
            pass

        # =======================================================
        # PHASE C1: k/v projections + elu, kv stats, allreduce, q
        # =======================================================
        if phases >= 2:
          with ExitStack() as ctx:
            wp = ctx.enter_context(tc.tile_pool(name="wC1", bufs=1))
            tmp = ctx.enter_context(tc.tile_pool(name="tmpC1", bufs=2))
            pkp = ctx.enter_context(tc.tile_pool(name="pkp", bufs=1))
            pk_sb = pkp.tile([128, SL // 128, H, HD], BF)  # pk token-major
            psB = ctx.enter_context(tc.tile_pool(name="psB_C1", bufs=2, space="PSUM"))
            psK = ctx.enter_context(tc.tile_pool(name="psK_C1", bufs=2, space="PSUM"))

            # ---- k -> pk (token-major, resident) ----
            wk_sb = wp.tile([128, ND, D], BF, tag="wbig")
            nc.sync.dma_start(out=wk_sb, in_=fm(wk))
            for ci in range(NCH):
                for sub in range(CH // 128):
                    blk = ci * (CH // 128) + sub
                    s0 = ci * CH + sub * 128
                    kps = psB.tile([128, D], F32, tag="psB")
                    for nh in range(2):
                        for kc in range(ND):
                            nc.tensor.matmul(kps[:, nh * 512:(nh + 1) * 512],
                                             xf_fm[:, kc, s0:s0 + 128],
                                             wk_sb[:, kc, nh * 512:(nh + 1) * 512],
                                             start=(kc == 0), stop=(kc == ND - 1))
                    tmin = tmp.tile([128, D], F32, tag="tmin")
                    nc.vector.tensor_scalar_min(tmin, kps, 0.0)
                    texp = tmp.tile([128, D], BF, tag="texp")
                    nc.scalar.activation(texp, tmin, AF.Exp)
                    trel = tmp.tile([128, D], BF, tag="trel")
                    nc.scalar.activation(trel, kps, AF.Relu)
                    nc.vector.tensor_add(
                        pk_sb[:, blk, :, :].rearrange("p h d -> p (h d)"),
                        texp, trel)

            # ---- v -> pv (chunk) + kv accumulation ----
            wv_sb = wp.tile([128, ND, D], BF, tag="wbig")
            nc.sync.dma_start(out=wv_sb, in_=fm(wv))
            for ci in range(NCH):
                pv_ch = tmp.tile([128, CH // 128, H, HD + 1], BF, tag="pv")
                nc.vector.memset(pv_ch[:, :, :, HD:HD + 1], 1.0)
                for sub in range(CH // 128):
                    s0 = ci * CH + sub * 128
                    vps = psB.tile([128, D], F32, tag="psB")
                    for nh in range(2):
                        for kc in range(ND):
                            nc.tensor.matmul(vps[:, nh * 512:(nh + 1) * 512],
                                             xf_fm[:, kc, s0:s0 + 128],
                                             wv_sb[:, kc, nh * 512:(nh + 1) * 512],
                                             start=(kc == 0), stop=(kc == ND - 1))
                    nc.vector.tensor_copy(
                        pv_ch[:, sub, :, 0:HD],
                        vps.rearrange("p (h d) -> p h d", h=H))
                for h in range(H):
                    kvp = psK.tile([64, HD + 1], F32, tag="psK")
                    for sub in range(CH // 128):
                        nc.tensor.matmul(
                            kvp,
                            pk_sb[:, ci * (CH // 128) + sub, h, :],
                            pv_ch[:, sub, h, :],
                            start=(sub == 0), stop=(sub == (CH // 128) - 1))
                    if ci == 0:
                        nc.vector.tensor_copy(kv_acc[:, h, :], kvp)
                    else:
                        nc.vector.tensor_add(kv_acc[:, h, :], kv_acc[:, h, :], kvp)

            # ---- allreduce kv over the core pair ----
            nc.sync.dma_start(
                out=kv_part, in_=kv_acc.rearrange("p g d -> p (g d)"))
            if timing:
                # NTFF profiling crashes on NEFFs with collectives through this
                # stack; the timing build substitutes a local copy (same data
                # volume, no cross-core sum) purely for profiling runs.
                nc.sync.dma_start(out=kv_sum, in_=kv_part)
            else:
                nc.gpsimd.collective_compute(
                    "AllReduce", mybir.AluOpType.add,
                    replica_groups=[[0, 1], [2, 3], [4, 5], [6, 7]],
                    ins=[kv_part], outs=[kv_sum])
            nc.sync.dma_start(
                out=kv_acc.rearrange("p g d -> p (g d)"), in_=kv_sum)
            nc.vector.tensor_copy(kv_bf, kv_acc)
            if debug:
                nc.sync.dma_start(out=dbg["dbg_kv"],
                                  in_=kv_acc.rearrange("p g d -> p (g d)"))

            # ---- q -> pq (feature-major, spilled) ----
            wq_sb = wp.tile([128, ND, D], BF, tag="wbig")
            nc.sync.dma_start(out=wq_sb, in_=fm(wq))
            for ci in range(NCH):
                c0 = ci * CH
                pq_ch = tmp.tile([128, ND, CH], BF, tag="pq")
                for mb in range(ND):
                    qps = psB.tile([128, CH], F32, tag="psB")
                    for kc in range(ND):
                        nc.tensor.matmul(qps, wq_sb[:, kc, mb * 128:(mb + 1) * 128],
                                         xf_fm[:, kc, c0:c0 + CH],
                                         start=(kc == 0), stop=(kc == ND - 1))
                    tmin = tmp.tile([128, CH], F32, tag="tminq")
                    nc.vector.tensor_scalar_min(tmin, qps, 0.0)
                    texp = tmp.tile([128, CH], BF, tag="texpq")
                    nc.scalar.activation(texp, tmin, AF.Exp)
                    trel = tmp.tile([128, CH], BF, tag="trelq")
                    nc.scalar.activation(trel, qps, AF.Relu)
                    nc.vector.tensor_add(pq_ch[:, mb, :], texp, trel)
                nc.sync.dma_start(out=fm(pq_d)[:, :, c0:c0 + CH], in_=pq_ch)
          xfp.release()
        else:
            xfp.release()

        # =======================================================
        # PHASE C2: linear attention + Wo + LN1 + LN0 -> h (fm)
        # =======================================================
        if phases >= 3:
          with ExitStack() as ctx:
            wp = ctx.enter_context(tc.tile_pool(name="wC2", bufs=1))
            wo_sb = wp.tile([128, ND, D], BF)
            nc.sync.dma_start(out=wo_sb, in_=fm(wo))
            n1g_b = bcast_vec(n1g, D, "n1g_b", wp)
            n1b_b = bcast_vec(n1b, D, "n1b_b", wp)
            io = ctx.enter_context(tc.tile_pool(name="ioC2", bufs=2))
            tmp = ctx.enter_context(tc.tile_pool(name="tmpC2", bufs=2))
            zt = ctx.enter_context(tc.tile_pool(name="ztC2", bufs=3))
            sm = ctx.enter_context(tc.tile_pool(name="smC2", bufs=4))
            psN = ctx.enter_context(tc.tile_pool(name="psN_C2", bufs=3, space="PSUM"))
            psB = ctx.enter_context(tc.tile_pool(name="psB_C2", bufs=1, space="PSUM"))
            psT = ctx.enter_context(tc.tile_pool(name="psT_C2", bufs=2, space="PSUM"))

            for ci in range(NCH):
                c0 = ci * CH
                # reload pq with heads split to partition-base 0: d = a*128+q*64+p
                pq_ch = io.tile([64, H, CH], BF, tag="pqi")
                nc.sync.dma_start(
                    out=pq_ch,
                    in_=pq_d.rearrange("(a q p) s -> p (a q) s",
                                       q=2, p=64)[:, :, c0:c0 + CH])
                xftm_ch = io.tile([128, CH // 128, D], F32, tag="xfi")
                nc.sync.dma_start(out=xftm_ch, in_=tmr(xf_tm_d[c0:c0 + CH, :]))
                hfm_ch = io.tile([128, ND, CH], BF, tag="hfm")
                x1_ch = io.tile([128, CH // 128, D], BF, tag="x1c")

                for sub in range(CH // 128):
                    s0 = sub * 128
                    # num/den for all 16 heads (4 per psum tile)
                    attn_tm = tmp.tile([128, D], BF, tag="attn")
                    for q4 in range(4):
                        nps = psN.tile([128, 4, HD + 1], F32, tag="psN")
                        for hh in range(4):
                            h = q4 * 4 + hh
                            nc.tensor.matmul(
                                nps[:, hh, :],
                                pq_ch[:, h, s0:s0 + 128],
                                kv_bf[:, h, :],
                                start=True, stop=True)
                        rd = sm.tile([128, 4], F32, tag="rd4")
                        nc.vector.tensor_scalar(rd, nps[:, :, HD], scalar1=EPS,
                                                scalar2=None, op0=AO.add)
                        nc.vector.reciprocal(rd, rd)
                        for hh in range(4):
                            h = q4 * 4 + hh
                            nc.vector.tensor_scalar_mul(
                                attn_tm[:, h * 64:(h + 1) * 64],
                                nps[:, hh, 0:HD], rd[:, hh:hh + 1])
                    # transpose attn -> fm
                    attn_fm = tmp.tile([128, ND, 128], BF, tag="attnf")
                    for db in range(ND):
                        tp = psT.tile([128, 128], BF, tag="psT")
                        nc.tensor.transpose(tp, attn_tm[:, db * 128:(db + 1) * 128],
                                            ident)
                        nc.vector.tensor_copy(attn_fm[:, db, :], tp)
                    # attn_out (token-major) + residual
                    ops = psB.tile([128, D], F32, tag="psB")
                    for nh in range(2):
                        for kc in range(ND):
                            nc.tensor.matmul(ops[:, nh * 512:(nh + 1) * 512],
                                             attn_fm[:, kc, :],
                                             wo_sb[:, kc, nh * 512:(nh + 1) * 512],
                                             start=(kc == 0), stop=(kc == ND - 1))
                    z = zt.tile([128, D], F32, tag="z")
                    nc.vector.tensor_add(z, ops, xftm_ch[:, sub, :])
                    # LN1
                    st = sm.tile([128, 2, 6], F32, tag="st")
                    for g2 in range(2):
                        nc.vector.bn_stats(st[:, g2, :], z[:, g2 * 512:(g2 + 1) * 512])
                    mv = sm.tile([128, 2], F32, tag="mv")
                    nc.vector.bn_aggr(mv, st)
                    rstd = sm.tile([128, 1], F32, tag="rstd")
                    nc.scalar.activation(rstd, mv[:, 1:2], AF.Sqrt, bias=eps5)
                    nc.vector.reciprocal(rstd, rstd)
                    u1 = zt.tile([128, D], F32, tag="u1")
                    nc.vector.tensor_scalar(u1, z, scalar1=mv[:, 0:1], scalar2=rstd,
                                            op0=AO.subtract, op1=AO.mult)
                    x1f = zt.tile([128, D], F32, tag="x1f")
                    nc.vector.scalar_tensor_tensor(out=x1f, in0=u1, scalar=0.0,
                                                   in1=n1g_b, op0=AO.add, op1=AO.mult)
                    nc.vector.tensor_add(x1f, x1f, n1b_b)
                    nc.scalar.copy(x1_ch[:, sub, :], x1f)
                    # LN0 -> h (affine folded into W1/b1)
                    st0 = sm.tile([128, 2, 6], F32, tag="st0")
                    for g2 in range(2):
                        nc.vector.bn_stats(st0[:, g2, :],
                                           x1f[:, g2 * 512:(g2 + 1) * 512])
                    mv0 = sm.tile([128, 2], F32, tag="mv0")
                    nc.vector.bn_aggr(mv0, st0)
                    rstd0 = sm.tile([128, 1], F32, tag="rstd0")
                    nc.scalar.activation(rstd0, mv0[:, 1:2], AF.Sqrt, bias=eps5)
                    nc.vector.reciprocal(rstd0, rstd0)
                    h_tm = zt.tile([128, D], BF, tag="h_tm")
                    nc.vector.tensor_scalar(h_tm, x1f, scalar1=mv0[:, 0:1],
                                            scalar2=rstd0,
                                            op0=AO.subtract, op1=AO.mult)
                    for db in range(ND):
                        tp = psT.tile([128, 128], BF, tag="psT")
                        nc.tensor.transpose(tp, h_tm[:, db * 128:(db + 1) * 128],
                                            ident)
                        nc.vector.tensor_copy(hfm_ch[:, db, s0:s0 + 128], tp)
                nc.sync.dma_start(out=fm(h_d)[:, :, c0:c0 + CH], in_=hfm_ch)
                nc.sync.dma_start(out=tmr(x1_d[c0:c0 + CH, :]), in_=x1_ch)
          kvp_pool.release()
        else:
            kvp_pool.release()

        if debug and phases >= 3:
            nc.gpsimd.dma_start(out=fm(dbg["dbg_h"]), in_=fm(h_d[:, :]))

        # =======================================================
        # PHASE C3a: FFN up-projection + gelu -> h1 (fm, spilled)
        # =======================================================
        if phases >= 4:
          with ExitStack() as ctx:
            wp = ctx.enter_context(tc.tile_pool(name="wC3a", bufs=1))
            w1_sb = wp.tile([128, ND, DFF], BF)
            nc.sync.dma_start(out=w1_sb, in_=fm(w1))
            io = ctx.enter_context(tc.tile_pool(name="ioC3a", bufs=2))
            h1p = ctx.enter_context(tc.tile_pool(name="h1C3a", bufs=1))
            psA = ctx.enter_context(tc.tile_pool(name="psA_C3a", bufs=3, space="PSUM"))

            for ci in range(NCH):
                c0 = ci * CH
                hfm_ch = io.tile([128, ND, CH], BF, tag="hin")
                nc.sync.dma_start(out=hfm_ch, in_=fm(h_d)[:, :, c0:c0 + CH])
                h1_sb = h1p.tile([128, NF, CH], BF, tag="h1")
                for fb in range(NF):
                    fps = psA.tile([128, CH], F32, tag="psA")
                    for kc in range(ND):
                        nc.tensor.matmul(fps, w1_sb[:, kc, fb * 128:(fb + 1) * 128],
                                         hfm_ch[:, kc, :],
                                         start=(kc == 0), stop=(kc == ND - 1))
                    nc.scalar.activation(h1_sb[:, fb, :], fps, AF.Gelu,
                                         bias=b1p_sb[:, fb:fb + 1])
                nc.sync.dma_start(out=fm(h1_d)[:, :, c0:c0 + CH], in_=h1_sb)

        # =======================================================
        # PHASE C3b: FFN down-projection + residual + LN2 -> y
        # =======================================================
        if phases >= 5:
          wcp = tc.alloc_tile_pool(name="wcp", bufs=1)
          wc_tm = wcp.tile([128, 2, D], F32)           # y[:256] for write phase
          with ExitStack() as ctx:
            wp = ctx.enter_context(tc.tile_pool(name="wC3b", bufs=1))
            w2_sb = wp.tile([128, NF, D], BF)
            nc.sync.dma_start(out=w2_sb, in_=fm(w2))
            n2g_b = bcast_vec(n2g, D, "n2g_b", wp)
            n2b_b = bcast_vec(n2b, D, "n2b_b", wp)
            b2_b = bcast_vec(b2v, D, "b2_b", wp)
            io = ctx.enter_context(tc.tile_pool(name="ioC3b", bufs=2))
            zt = ctx.enter_context(tc.tile_pool(name="ztC3b", bufs=2))
            sm = ctx.enter_context(tc.tile_pool(name="smC3b", bufs=4))
            psB = ctx.enter_context(tc.tile_pool(name="psB_C3b", bufs=2, space="PSUM"))

            for ci in range(NCH):
                c0 = ci * CH
                h1_sb = io.tile([128, NF, CH], BF, tag="h1in", bufs=1)
                nc.sync.dma_start(out=h1_sb, in_=fm(h1_d)[:, :, c0:c0 + CH])
                x1_ch = io.tile([128, CH // 128, D], BF, tag="x1in")
                nc.sync.dma_start(out=x1_ch, in_=tmr(x1_d[c0:c0 + CH, :]))
                for sub in range(CH // 128):
                    s0 = sub * 128
                    f2 = psB.tile([128, D], F32, tag="psB")
                    for nh in range(2):
                        for kc in range(NF):
                            nc.tensor.matmul(f2[:, nh * 512:(nh + 1) * 512],
                                             h1_sb[:, kc, s0:s0 + 128],
                                             w2_sb[:, kc, nh * 512:(nh + 1) * 512],
                                             start=(kc == 0), stop=(kc == NF - 1))
                    s2 = zt.tile([128, D], F32, tag="s2")
                    nc.vector.scalar_tensor_tensor(
                        out=s2, in0=f2, scalar=0.0,
                        in1=x1_ch[:, sub, :],
                        op0=AO.add, op1=AO.add)
                    nc.vector.tensor_add(s2, s2, b2_b)
                    # LN2
                    st = sm.tile([128, 2, 6], F32, tag="st")
                    for g2 in range(2):
                        nc.vector.bn_stats(st[:, g2, :],
                                           s2[:, g2 * 512:(g2 + 1) * 512])
                    mv = sm.tile([128, 2], F32, tag="mv")
                    nc.vector.bn_aggr(mv, st)
                    rstd = sm.tile([128, 1], F32, tag="rstd")
                    nc.scalar.activation(rstd, mv[:, 1:2], AF.Sqrt, bias=eps5)
                    nc.vector.reciprocal(rstd, rstd)
                    u2 = zt.tile([128, D], F32, tag="u2")
                    nc.vector.tensor_scalar(u2, s2, scalar1=mv[:, 0:1], scalar2=rstd,
                                            op0=AO.subtract, op1=AO.mult)
                    ysb = zt.tile([128, D], F32, tag="ysb")
                    nc.vector.scalar_tensor_tensor(out=ysb, in0=u2, scalar=0.0,
                                                   in1=n2g_b, op0=AO.add, op1=AO.mult)
                    nc.vector.tensor_add(ysb, ysb, n2b_b)
                    r0 = c0 + s0
                    nc.sync.dma_start(out=y_out[r0:r0 + 128, :], in_=ysb)
                    if ci == 0 and sub < 2:
                        nc.vector.tensor_copy(wc_tm[:, sub, :], ysb)

        # =======================================================
        # PHASE W: slot-routed cache write (even cores' result used)
        # =======================================================
        if phases >= 6:
          with ExitStack() as ctx:
            wp = ctx.enter_context(tc.tile_pool(name="wW", bufs=1))
            wslot_sb = wp.tile([128, ND, DC], BF)
            nc.sync.dma_start(out=wslot_sb, in_=fm(wslot))
            wvw_sb = wp.tile([128, ND, DC], BF)
            nc.sync.dma_start(out=wvw_sb, in_=fm(wvw))
            wgw_sb = wp.tile([128, ND, 1], BF)
            nc.sync.dma_start(out=wgw_sb, in_=fm(wgw))
            lT_sb = wp.tile([128, NC4, K], BF)
            nc.sync.dma_start(out=lT_sb, in_=fm(localT))
            lf_sb = wp.tile([K, DC], F32)
            nc.sync.dma_start(out=lf_sb, in_=local_f)

            tmp = ctx.enter_context(tc.tile_pool(name="tmpW", bufs=2))
            sm = ctx.enter_context(tc.tile_pool(name="smW", bufs=4))
            psA = ctx.enter_context(tc.tile_pool(name="psA_W", bufs=2, space="PSUM"))
            psU = ctx.enter_context(tc.tile_pool(name="psU_W", bufs=1, space="PSUM"))
            psT = ctx.enter_context(tc.tile_pool(name="psT_W", bufs=2, space="PSUM"))

            # wc feature-major
            wc_fm = tmp.tile([128, ND, T], BF, tag="wcfm")
            for sub in range(2):
                wcb = tmp.tile([128, D], BF, tag="wcb")
                nc.scalar.copy(wcb, wc_tm[:, sub, :])
                for db in range(ND):
                    tp = psT.tile([128, 128], BF, tag="psT")
                    nc.tensor.transpose(tp, wcb[:, db * 128:(db + 1) * 128], ident)
                    nc.vector.tensor_copy(wc_fm[:, db, sub * 128:(sub + 1) * 128], tp)

            # slot_in^T = Wslot^T @ wc^T  [DC, T]
            si_sb = tmp.tile([128, NC4, T], BF, tag="si")
            for cb in range(NC4):
                ps = psA.tile([128, T], F32, tag="psA")
                for kc in range(ND):
                    nc.tensor.matmul(ps, wslot_sb[:, kc, cb * 128:(cb + 1) * 128],
                                     wc_fm[:, kc, :],
                                     start=(kc == 0), stop=(kc == ND - 1))
                nc.scalar.copy(si_sb[:, cb, :], ps)

            # logits^T = local @ slot_in^T  [K, T] -> exp
            lps = psA.tile([64, T], F32, tag="psA")
            for cc in range(NC4):
                nc.tensor.matmul(lps, lT_sb[:, cc, :], si_sb[:, cc, :],
                                 start=(cc == 0), stop=(cc == NC4 - 1))
            el_sb = tmp.tile([64, T], BF, tag="el")
            nc.scalar.activation(el_sb, lps, AF.Exp, scale=ISQ)

            # per-token gate*recip(den); gated probs (token-major)
            gp_sb = tmp.tile([128, 2, K], BF, tag="gp")
            for sub in range(2):
                s0 = sub * 128
                dps = psU.tile([128, 1], F32, tag="psUd")
                nc.tensor.matmul(dps, el_sb[:, s0:s0 + 128], ones_bf[0:64, :],
                                 start=True, stop=True)
                gps = psU.tile([128, 1], F32, tag="psUd")
                for kc in range(ND):
                    nc.tensor.matmul(gps, wc_fm[:, kc, s0:s0 + 128],
                                     wgw_sb[:, kc, :],
                                     start=(kc == 0), stop=(kc == ND - 1))
                rd = sm.tile([128, 1], F32, tag="rdw")
                nc.vector.reciprocal(rd, dps)
                sg = sm.tile([128, 1], F32, tag="sgw")
                nc.scalar.activation(sg, gps, AF.Sigmoid, bias=bgw_t)
                gw = sm.tile([128, 1], F32, tag="gww")
                nc.vector.tensor_mul(gw, rd, sg)
                tp = psT.tile([128, 64], BF, tag="psT")
                nc.tensor.transpose(tp, el_sb[:, s0:s0 + 128], ident[0:64, 0:64])
                nc.vector.tensor_scalar_mul(gp_sb[:, sub, :], tp, gw)

            # vals (token-major)
            vals_sb = tmp.tile([128, 2, DC], BF, tag="vals")
            for sub in range(2):
                s0 = sub * 128
                vps = psA.tile([128, DC], F32, tag="psA")
                for kc in range(ND):
                    nc.tensor.matmul(vps, wc_fm[:, kc, s0:s0 + 128],
                                     wvw_sb[:, kc, :],
                                     start=(kc == 0), stop=(kc == ND - 1))
                nc.scalar.copy(vals_sb[:, sub, :], vps)

            # update = (gated probs)^T @ vals ; new_local = local + update
            ups = psU.tile([64, DC], F32, tag="psUu")
            for sub in range(2):
                nc.tensor.matmul(ups, gp_sb[:, sub, :], vals_sb[:, sub, :],
                                 start=(sub == 0), stop=(sub == 1))
            nl_sb = tmp.tile([64, DC], F32, tag="nl")
            nc.vector.tensor_add(nl_sb, ups, lf_sb)
            nc.sync.dma_start(out=nl_out, in_=nl_sb)
          wcp.release()
        if phases == 5:
            wcp.release()

    nc.compile()
    return nc


def _get(debug=False, phases=6, timing=False):
    key = (debug, phases, timing)
    if key not in _CACHE:
        _CACHE[key] = _build(debug, phases, timing)
    return _CACHE[key]


def _prep_inputs(inputs):
    bf16 = ml_dtypes.bfloat16
    f32 = np.float32

    def nf(name):
        return np.ascontiguousarray(np.asarray(inputs[name], dtype=f32))

    x = nf("x")
    cache = nf("cache")
    W1 = nf("W1")
    ln0_g = nf("ln0_g")
    ln0_b = nf("ln0_b")
    w1p = (ln0_g[:, None] * W1)
    b1p = (nf("b1") + ln0_b @ W1).astype(f32)

    def b(a):  # bf16 cast, contiguous
        return np.ascontiguousarray(a.astype(bf16))

    shared = {
        "wqr": b(nf("Wq_r")), "wor": b(nf("Wo_r")), "wgr": b(nf("Wg_r")),
        "wq": b(nf("Wq")), "wk": b(nf("Wk")), "wv": b(nf("Wv")),
        "wo": b(nf("Wo")), "w1": b(w1p), "w2": b(nf("W2")),
        "wslot": b(nf("Wslot")), "wvw": b(nf("Wvw")), "wgw": b(nf("Wgw")),
        "bg_r": nf("bg_r"), "bgw": nf("bgw"), "b1p": b1p, "b2v": nf("b2"),
        "n1g": nf("n1_g"), "n1b": nf("n1_b"),
        "n2g": nf("n2_g"), "n2b": nf("n2_b"),
    }
    in_maps = []
    for c in range(N_CORES):
        bi, hi = c // 2, c % 2
        xs = x[bi, hi * SL:(hi + 1) * SL]          # [SL, D]
        cb = cache[bi]                              # [M, DC]
        loc = cb[LAYER * K:(LAYER + 1) * K]         # [K, DC]
        m = dict(shared)
        m["x_tm"] = np.ascontiguousarray(xs)
        m["x_fm"] = b(xs.T)
        m["cacheT"] = b(cb.T)
        m["cache_m"] = b(cb)
        m["localT"] = b(loc.T)
        m["local_f"] = np.ascontiguousarray(loc)
        in_maps.append(m)
    return in_maps


def kernel(**inputs):
    from concourse.bass_utils import run_bass_kernel_spmd
    nc = _get()
    in_maps = _prep_inputs(inputs)
    res = run_bass_kernel_spmd(nc, in_maps, list(range(N_CORES)))
    return _assemble(inputs, res.results)


def _assemble(inputs, results):
    f32 = np.float32
    y = np.empty((B, S, D), dtype=f32)
    for c in range(N_CORES):
        bi, hi = c // 2, c % 2
        y[bi, hi * SL:(hi + 1) * SL] = results[c]["y"]
    new_cache = np.array(np.asarray(inputs["cache"], dtype=f32), copy=True)
    for bi in range(B):
        new_cache[bi, LAYER * K:(LAYER + 1) * K] = results[2 * bi]["new_local"]
    return y, new_cache
